# revision 35
# baseline (speedup 1.0000x reference)
"""Trainium2 Bass kernel for BinaryHead: logits = (l2norm(fea) @ W.T + b) * 16.

Sharding: data-parallel over the batch dim across 8 NeuronCores (2048 rows
each).  The host stages each core's shard TRANSPOSED ([emb, batch], a layout
choice) so the embedding/contraction dim lands on SBUF partitions, which is
what the TensorEngine contracts over.  Per core the device kernel streams
e-panel pairs [256e x 2048b]:

  z.T[c, b]   += Wt_chunk.T @ panel            (4-col stationary, panel moving)
  sumsq[b]    += ones.T @ panel**2             (squares on ACT/DVE)

and a small epilogue computes out.T = z.T * (S/sqrt(sumsq)) + S*b on device
(rsqrt via exp(-0.5*ln(ss)+ln(S)) on the scalar engine, class-broadcast via a
k=1 matmul).  The normalization never touches the big tensor.  The first and
last pairs are delivered in four column chunks: the first so the PE starts
early, the last so the epilogue pipelines into the tail of the stream.

Two configs:
  bf16 (default): shard staged as bf16 (halves HBM traffic), z matmuls in
      bf16, sumsq via fp8e4m3 DoubleRow matmuls (one MM contracts both
      panels).  resid-var style error ~2e-6 (scale-relative absmax ~2e-3).
  fp32: full-precision staging streamed as float32r (single-pass PE mode),
      bf16 squares.  scale-relative absmax ~1.1e-4, slower (HBM bound).
"""

import os
from contextlib import ExitStack

import numpy as np

NUM_CLASS = 4
EMB = 2048
BATCH = 16384
N_CORES = 8
ROWS = BATCH // N_CORES  # 2048 rows per core
S = 16.0

N_ETILES = EMB // 128  # 16 e-panels per core
N_BCHUNK = ROWS // 512  # 4 psum-width chunks of the batch

# compute dtype config: "bf16" (fast, default) or "fp32" (f32r matmuls)
DTYPE_CFG = os.environ.get("KERNEL_DTYPE", "bf16")

_CACHE = {}


def _build_nc():
    import concourse.bacc as bacc
    import concourse.mybir as mybir
    import concourse.tile as tile
    from concourse.hw_specs import get_activation_tables

    f32 = mybir.dt.float32
    f32r = mybir.dt.float32r
    bf16 = mybir.dt.bfloat16
    fp8 = mybir.dt.float8e4
    use_bf16 = DTYPE_CFG == "bf16"
    dt_data = bf16 if use_bf16 else f32r
    dt_sq = fp8 if use_bf16 else bf16

    nc = bacc.Bacc(
        "TRN2",
        target_bir_lowering=False,
        debug=False,
        enable_asserts=False,
        num_devices=N_CORES,
    )

    feaT = nc.dram_tensor("feaT", [EMB, ROWS], dt_data, kind="ExternalInput").ap()
    wt = nc.dram_tensor(
        "wt", [128, N_ETILES * NUM_CLASS], dt_data, kind="ExternalInput"
    ).ap()
    if use_bf16:
        onesv = nc.dram_tensor("onesv", [128, 2, 16], fp8, kind="ExternalInput").ap()
    else:
        onesv = nc.dram_tensor("onesv", [128, 1], dt_sq, kind="ExternalInput").ap()
    sones = nc.dram_tensor("sones", [1, NUM_CLASS], f32r, kind="ExternalInput").ap()
    sbias = nc.dram_tensor("sbias", [NUM_CLASS, 1], f32, kind="ExternalInput").ap()
    outT = nc.dram_tensor("outT", [NUM_CLASS, ROWS], f32, kind="ExternalOutput").ap()

    with tile.TileContext(nc) as tc, ExitStack() as ctx:
        pconst = ctx.enter_context(tc.tile_pool(name="pconst", bufs=1))
        pdata = ctx.enter_context(tc.tile_pool(name="pdata", bufs=7))
        psq = ctx.enter_context(tc.tile_pool(name="psq", bufs=6))
        pep = ctx.enter_context(tc.tile_pool(name="pep", bufs=1))
        pz = ctx.enter_context(tc.tile_pool(name="pz", bufs=1, space="PSUM"))
        ps = ctx.enter_context(tc.tile_pool(name="ps", bufs=1, space="PSUM"))

        # wt/ones ride the front of the sync ring (tiny transfers that the
        # first matmuls need); the tail-only consts go through SWDGE
        wt_s = pconst.tile([128, N_ETILES * NUM_CLASS], dt_data)
        nc.sync.dma_start(out=wt_s, in_=wt)
        if use_bf16:
            ones_s = pconst.tile([128, 2, 16], fp8)
        else:
            ones_s = pconst.tile([128, 1], dt_sq)
        nc.sync.dma_start(out=ones_s, in_=onesv)
        sones_s = pconst.tile([1, NUM_CLASS], f32r)
        nc.gpsimd.dma_start(out=sones_s, in_=sones)
        sbias_s = pconst.tile([NUM_CLASS, 1], f32)
        nc.gpsimd.dma_start(out=sbias_s, in_=sbias)
        zero1_s = pconst.tile([1, 1], f32)
        nc.vector.memset(zero1_s, 0.0)
        zero128_s = pconst.tile([128, 1], f32)
        nc.vector.memset(zero128_s, 0.0)
        # rsqrt via exp(-0.5*ln(ss) + ln(S)): folds the *S scale in for free
        lnS_s = pconst.tile([1, 1], f32)
        nc.vector.memset(lnS_s, float(np.log(S)))

        # accumulators: z.T as one 4-bank tensor (PE-only writers), sumsq as
        # four single-bank tensors so the epilogue psum reuse pipelines
        zt_ps = pz.tile([NUM_CLASS, ROWS], f32, tag="zt")
        ss_ps = [
            ps.tile([1, 512], f32, tag="ssrnb", bufs=4, name=f"ss{j}")
            for j in range(N_BCHUNK)
        ]
        rnb = [
            ps.tile([NUM_CLASS, 512], f32, tag="ssrnb", bufs=4, name=f"rnb{j}")
            for j in range(N_BCHUNK)
        ]
        lnss_s = pep.tile([1, ROWS], f32)
        rnorm_s = pep.tile([1, ROWS], f32r)
        z_s = pep.tile([NUM_CLASS, ROWS], f32)
        zr_s = pep.tile([NUM_CLASS, ROWS], f32)
        out_s = pep.tile([NUM_CLASS, ROWS], f32)

        def square(a, bsl, xt, x2):
            # panel 0 on ACT, panel 1 on DVE: they run concurrently
            xin = xt[:, a, bsl] if use_bf16 else xt[:, a, bsl].bitcast(f32)
            if a == 0:
                nc.scalar.activation(
                    out=x2[:, a, bsl],
                    in_=xin,
                    func=mybir.ActivationFunctionType.Square,
                    bias=zero128_s,
                    scale=1.0,
                )
            else:
                nc.vector.tensor_mul(x2[:, a, bsl], xin, xin)

        def z_mm(t, j, xt, a, start, stop):
            bsl = slice(j * 512, (j + 1) * 512)
            nc.tensor.matmul(
                zt_ps[:, bsl],
                wt_s[:, t * NUM_CLASS : (t + 1) * NUM_CLASS],
                xt[:, a, bsl],
                start=start,
                stop=stop,
            )

        def ss_mm(j, x2, start, stop):
            bsl = slice(j * 512, (j + 1) * 512)
            if use_bf16:
                # fp8 DoubleRow: one matmul contracts both panels (k=256)
                nc.tensor.matmul(
                    ss_ps[j],
                    ones_s[:, :, 0:1],
                    x2[:, :, bsl],
                    perf_mode=mybir.MatmulPerfMode.DoubleRow,
                    start=start,
                    stop=stop,
                )
            else:
                for a in range(2):
                    nc.tensor.matmul(
                        ss_ps[j], ones_s, x2[:, a, bsl], start=start, stop=stop
                    )

        def epilogue_chunk(j):
            # out.T[c,b] = z.T[c,b] * S/sqrt(sumsq[b]) + S*bias[c]
            bsl = slice(j * 512, (j + 1) * 512)
            nc.vector.tensor_copy(z_s[:, bsl], zt_ps[:, bsl])
            nc.scalar.activation(
                out=lnss_s[:, bsl],
                in_=ss_ps[j],
                func=mybir.ActivationFunctionType.Ln,
                bias=zero1_s,
                scale=1.0,
            )
            nc.scalar.activation(
                out=rnorm_s[:, bsl],
                in_=lnss_s[:, bsl],
                func=mybir.ActivationFunctionType.Exp,
                bias=lnS_s,
                scale=-0.5,
            )
            # broadcast S/norm across the 4 class partitions via a k=1 f32r
            # matmul (single-pass PE; reuses a freed sumsq psum bank)
            nc.tensor.matmul(rnb[j], sones_s, rnorm_s[:, bsl], start=True, stop=True)
            nc.vector.tensor_mul(zr_s[:, bsl], z_s[:, bsl], rnb[j])
            nc.vector.tensor_scalar_add(
                out_s[:, bsl], in0=zr_s[:, bsl], scalar1=sbias_s
            )
            nc.sync.dma_start(out=outT[:, bsl], in_=out_s[:, bsl])

        # pre-warm the PE while the first data transfer is in flight: dummy
        # matmuls into zt_ps (the first real z matmul's start=True resets the
        # bank, so the garbage never survives).  Keeps the HAM clock-gate at
        # full rate when real matmuls begin.
        for _ in range(24):
            nc.tensor.matmul(
                zt_ps[:, 0:64],
                wt_s[:, 0:NUM_CLASS],
                wt_s[:, 0:64],
                start=True,
                stop=True,
            )

        pairs = [(t, t + 1) for t in range(0, N_ETILES, 2)]
        for gi, g in enumerate(pairs):
            first = gi == 0
            last = gi == len(pairs) - 1
            xt = pdata.tile([128, 2, ROWS], dt_data, tag="xt")
            x2 = psq.tile([128, 2, ROWS], dt_sq, tag="x2")
            src = feaT[g[0] * 128 : (g[1] + 1) * 128, :].rearrange(
                "(a p) b -> p a b", p=128
            )
            # alternate the two HWDGE rings (SP and ACT) so transfers overlap
            dma_eng = nc.sync if gi % 2 == 0 else nc.scalar
            if first or last:
                # column-chunked delivery: first pair lets the PE start after
                # a quarter transfer; last pair lets the epilogue overlap the
                # stream tail
                for j in range(N_BCHUNK):
                    bsl = slice(j * 512, (j + 1) * 512)
                    dma_eng.dma_start(out=xt[:, :, bsl], in_=src[:, :, bsl])
            else:
                dma_eng.dma_start(out=xt, in_=src)
            if gi == 1:
                # preload the one ACT table set covering Square+Ln+Exp so no
                # table switch ever lands on the critical path
                nlx_id = list(get_activation_tables(nc.m.arch)).index(
                    "natural_log_exp_and_others"
                )
                nc.scalar.add_instruction(
                    mybir.InstLoadActFuncSet(
                        name=f"I-{nc.next_id()}", act_func_set_id=nlx_id
                    )
                )

            if first or last:
                # per-chunk squares + matmuls so chunk j's chain completes
                # without waiting for the whole pair
                for j in range(N_BCHUNK):
                    bsl = slice(j * 512, (j + 1) * 512)
                    square(0, bsl, xt, x2)
                    square(1, bsl, xt, x2)
                    if last:
                        ss_mm(j, x2, start=first, stop=last)
                        z_mm(g[0], j, xt, 0, start=first, stop=False)
                        z_mm(g[1], j, xt, 1, start=False, stop=last)
                        epilogue_chunk(j)
                    else:
                        z_mm(g[0], j, xt, 0, start=first, stop=False)
                        z_mm(g[1], j, xt, 1, start=False, stop=last)
                        ss_mm(j, x2, start=first, stop=last)
            else:
                full = slice(None)
                square(0, full, xt, x2)
                square(1, full, xt, x2)
                # z matmuls first (need only xt), ss after (needs squares)
                for a in range(2):
                    for j in range(N_BCHUNK):
                        z_mm(g[a], j, xt, a, start=False, stop=False)
                for j in range(N_BCHUNK):
                    ss_mm(j, x2, start=False, stop=False)

    nc.compile()
    return nc


def _get_nc():
    if "nc" not in _CACHE:
        _CACHE["nc"] = _build_nc()
    return _CACHE["nc"]


def _stage_inputs(fea, W, b):
    import ml_dtypes

    np_data = ml_dtypes.bfloat16 if DTYPE_CFG == "bf16" else np.float32
    fea = np.asarray(fea, dtype=np.float32)
    W = np.asarray(W, dtype=np.float32)
    b = np.asarray(b, dtype=np.float32)

    # wt[p, 4t+c] = W[c, 128t+p]
    wt = np.ascontiguousarray(
        W.reshape(NUM_CLASS, N_ETILES, 128).transpose(2, 1, 0).reshape(128, -1)
    ).astype(np_data)
    if DTYPE_CFG == "bf16":
        onesv = np.zeros((128, 2, 16), dtype=ml_dtypes.float8_e4m3)
        onesv[:, :, 0] = 1.0
    else:
        onesv = np.ones((128, 1), dtype=ml_dtypes.bfloat16)
    # the *S scale is folded into the exp(-0.5*ln(ss)+ln(S)) rsqrt, so the
    # class-broadcast matmul uses plain ones
    sones = np.ones((1, NUM_CLASS), dtype=np.float32)
    sbias = (S * b).reshape(NUM_CLASS, 1).astype(np.float32)

    in_maps = []
    for i in range(N_CORES):
        shard = fea[i * ROWS : (i + 1) * ROWS, :]
        feaT = np.ascontiguousarray(shard.T).astype(np_data)
        in_maps.append(
            {"feaT": feaT, "wt": wt, "onesv": onesv, "sones": sones, "sbias": sbias}
        )
    return in_maps


def run(fea, W, b, trace=False):
    from concourse.bass_utils import run_bass_kernel_spmd

    nc = _get_nc()
    in_maps = _stage_inputs(fea, W, b)
    res = run_bass_kernel_spmd(nc, in_maps, core_ids=list(range(N_CORES)), trace=trace)
    out = np.empty((BATCH, NUM_CLASS), dtype=np.float32)
    for i in range(N_CORES):
        out[i * ROWS : (i + 1) * ROWS, :] = res.results[i]["outT"].T
    return out, res


def kernel(fea, W, b):
    out, _ = run(fea, W, b, trace=False)
    return out
